# revision 88
# baseline (speedup 1.0000x reference)
"""Trainium2 Bass kernel for nn_DeepNNDendroMatrix.

Math (reference):
    cols = path_mat[:, node_idx]                       # (E, B) in {0,1}
    layer(h, root, delta): relu(h @ root + sum_e cols[e,b] * (h @ W_e))
        where W_e[i,o] = delta[o,i,e]
    out = squeeze(layer2(layer1(x)))

Factorization used here (avoids the (B,in,out) intermediate):
    h1[b,o] = relu( (x@root0)[b,o] + sum_e colsT[b,e] * (x @ W_e)[b,o] )
    out[b]  = relu( sum_h h1[b,h] * wt[b,h] )
        wt[b,h] = root1[h] + sum_e colsT[b,e]*delta1[0,h,e]  (computed up
        front on the PE as colsE^T @ delta1[0]^T + ones @ root1^T)

Distribution: data-parallel over batch. 8 cores x 256 samples. Each core
streams the full (rearranged, bf16) delta0 once from HBM (33.5 MB), keeps
x^T resident in SBUF as the matmul stationary operand, accumulates the
per-edge scaled matmul outputs on the vector engine with fused
scalar_tensor_tensor (acc = psum_e * colsT[:,e] + acc).

The main PE stream (64 e-pairs x 2 batch tiles x 4 K-chunks x 512-col
matmuls) is the hard floor at ~109.2us; the schedule is tuned so the PE
runs it back-to-back:
  - all DMA sources are host-pre-tiled so every descriptor is a full
    >=512B row (no read-modify-write penalty), and the issue order is
    tuned against the serialized HWDGE descriptor-gen (625ns each) and
    DMA-engine (360B/ns) stages so each e-pair lands just before the PE
    needs it;
  - NWARM warm-up matmuls on a zeroed tile (plus NFILL1 fillers after
    ep0) keep the PE clock ramp hot until real data arrives;
  - x@root0 is evacuated straight into the accumulator (no separate
    memset/add) and root/wts matmuls are emitted early so the scheduler
    sinks them into DMA-wait gaps;
  - for the tail, the last e-pairs skip the ACT evacuation (DVE reads
    the PSUM halves directly) and the final e-pair accumulates each edge
    in its own PSUM bank so the first STT starts 4 matmuls early; the
    final relu runs on the DVE and a single combined DMA writes both
    batch tiles' outputs.
"""

import numpy as np
import ml_dtypes

import concourse.bass as bass
import concourse.mybir as mybir
from concourse.tile import TileContext
from concourse.bass_utils import run_bass_kernel_spmd

# ---------------------------------------------------------------------------
# Workaround: this walrus build allows only ONE sync wait per CTRL (Drain)
# instruction; TileContext's tail drain aggregates one wait per live
# semaphore onto a single Drain. Split them across multiple Drains.
import bass_rust
import concourse.tile as _tile_mod
from concourse.vector_clock import ScopedClock as _ScopedClock

_MAX_WAITS_PER_INST = 1


def _split_drain_and_barrier(self, tick_clock, wait_clock):
    nc = self.nc
    drain_inst = nc.sync.drain()
    wait_clock.add_sem_waits(
        drain_inst.ins, _ScopedClock({None: tick_clock.global_clock})
    )
    si = drain_inst.ins.sync_info
    waits = list(si.on_wait) if si is not None else []
    # DMA-queue sems complete last (out-DMA + 900ns prop): drain them after
    # the engine sems so those drains retire while the DMA is in flight
    waits.sort(key=lambda w: (w.ant_name or "").startswith(("DMAHW", "DMASW")))
    if len(waits) > _MAX_WAITS_PER_INST:
        si.on_wait = waits[:_MAX_WAITS_PER_INST]
        rest = waits[_MAX_WAITS_PER_INST:]
        for i in range(0, len(rest), _MAX_WAITS_PER_INST):
            extra = nc.sync.drain()
            chunk = rest[i : i + _MAX_WAITS_PER_INST]
            esi = extra.ins.sync_info
            if esi is None:
                extra.ins.sync_info = bass_rust.SyncInfo(on_wait=chunk, on_update=[])
            else:
                esi.on_wait = list(esi.on_wait) + chunk
    nc.all_engine_barrier()
    assert self.sems is not None
    popped = nc._tile_sem_poison_stack.pop()
    assert popped is self._sem_poison
    nc.clear_and_free_semaphores(list(self.sems.allocated().values()))
    nc.all_engine_barrier()


_tile_mod.TileContext._drain_and_barrier = _split_drain_and_barrier


_COALESCE_OK = {"Ldweights", "NoOp", "TensorCopy", "Memset", "TensorScalarPtr",
                "Matmult", "Activation", "TensorScalar"}


import os as _os2

_WAIT_CAP_DEFAULT = int(_os2.environ.get("KW_WAIT_CAP", "1"))


def _legalize_wait_counts(nc, max_waits=None):
    """Split any instruction carrying more than `max_waits` sync waits.

    Moving a wait onto an earlier instruction of the same engine is always
    safe (the engine just blocks earlier), so first try to coalesce excess
    waits onto the immediately-preceding same-engine instruction if it has
    spare wait slots; otherwise insert a NoOp carrying the wait."""
    if max_waits is None:
        max_waits = _WAIT_CAP_DEFAULT
    n_nops = 0
    for f in nc.m.functions:
        for bb in f.blocks:
            out = []
            for inst in bb.instructions:
                si = inst.sync_info
                waits = list(si.on_wait) if si is not None else []
                if len(waits) > max_waits:
                    si.on_wait = waits[:max_waits]
                    rest = waits[max_waits:]
                    # try to place excess on the immediately-preceding
                    # same-engine instruction (moving a wait earlier on the
                    # same engine is always safe, as long as that instruction
                    # does not itself update the awaited semaphore)
                    if out:
                        prev = out[-1]
                        if prev.engine == inst.engine and prev.opcode in _COALESCE_OK:
                            psi = prev.sync_info
                            pw = list(psi.on_wait) if psi is not None else []
                            upd_ids = {
                                u.id
                                for u in (psi.on_update if psi is not None else [])
                            }
                            while (
                                rest
                                and len(pw) < max_waits
                                and rest[0].id not in upd_ids
                            ):
                                pw.append(rest.pop(0))
                            if pw:
                                if psi is None:
                                    prev.sync_info = bass_rust.SyncInfo(
                                        on_wait=pw, on_update=[]
                                    )
                                else:
                                    psi.on_wait = pw
                    for i in range(0, len(rest), max_waits):
                        nop = bass_rust.InstNoOp(
                            name=f"{inst.name}-ws{i}", engine=inst.engine,
                            ins=[], outs=[],
                        )
                        nop.sync_info = bass_rust.SyncInfo(
                            on_wait=rest[i : i + max_waits], on_update=[]
                        )
                        out.append(nop)
                        n_nops += 1
                out.append(inst)
            bb.instructions = out
    return n_nops
# ---------------------------------------------------------------------------

# ---------------------------------------------------------------------------
# Persistent NEFF cache: walrus compilation of this kernel takes minutes and
# bass2jax recompiles per process. Cache the compiled NEFF on disk keyed by
# the BIR sha256 so repeat processes skip the compile.
import hashlib as _hashlib
import os as _os
import shutil as _shutil

import concourse.bass2jax as _bass2jax
import concourse.bass_utils as _bass_utils_mod

_NEFF_CACHE_DIR = _os.path.expanduser("~/.cache/bass_neff")
_orig_compile_bir_kernel = _bass_utils_mod.compile_bir_kernel


def _bir_cache_key(raw: bytes) -> str:
    """sha256 of the BIR with all debug info stripped: ant_debug blobs embed
    full tracebacks (including the CALLER's file/line), which vary with the
    directory and script kernel.py is invoked from."""
    import orjson

    d = orjson.loads(raw)
    d.pop("debug_table", None)

    def scrub(o):
        if isinstance(o, dict):
            o.pop("ant_debug", None)
            o.pop("debug", None)
            for v in o.values():
                scrub(v)
        elif isinstance(o, list):
            for v in o:
                scrub(v)

    scrub(d)
    return _hashlib.sha256(orjson.dumps(d)).hexdigest()


def _source_cache_key():
    """Fallback key: hash of this file's exact contents (+ env knobs that
    affect codegen). The builder is deterministic given the source, and a
    NEFF compiled in one calling context is valid in any other (tensor
    bindings are positional), so this key is safe when the BIR-byte key
    misses due to residual context-dependent debug noise."""
    with open(__file__, "rb") as f:
        src = f.read()
    return _hashlib.sha256(
        src + str(_WAIT_CAP_DEFAULT).encode()
    ).hexdigest()


def _cached_compile_bir_kernel(bir_json, tmpdir, neff_name="file.neff"):
    try:
        raw = bir_json if isinstance(bir_json, bytes) else bir_json.encode()
        keys = [_bir_cache_key(raw), "src" + _source_cache_key()]
        cpaths = [
            _os.path.join(_NEFF_CACHE_DIR, f"{k}_{neff_name}") for k in keys
        ]
        for cpath in cpaths:
            if _os.path.exists(cpath):
                dst = _os.path.join(tmpdir, "sg00")
                _os.makedirs(dst, exist_ok=True)
                dst_neff = _os.path.join(dst, neff_name)
                _shutil.copy(cpath, dst_neff)
                return dst_neff
    except Exception:
        return _orig_compile_bir_kernel(bir_json, tmpdir, neff_name)
    neff_path = _orig_compile_bir_kernel(bir_json, tmpdir, neff_name)
    try:
        _os.makedirs(_NEFF_CACHE_DIR, exist_ok=True)
        for cpath in cpaths:
            tmp = cpath + ".tmp"
            _shutil.copy(neff_path, tmp)
            _os.replace(tmp, cpath)
    except Exception:
        pass
    return neff_path


_bass2jax.compile_bir_kernel = _cached_compile_bir_kernel
_bass_utils_mod.compile_bir_kernel = _cached_compile_bir_kernel
# ---------------------------------------------------------------------------

NCORES = 8
B, F, H, O, E, N_NODES = 2048, 512, 256, 1, 128, 4096
BL = B // NCORES          # samples per core = 256
NBT = BL // 128           # b-tiles per core = 2
EP = E // 2               # e-pairs = 64
KI = F // 128             # contraction chunks over input features = 4

F32 = mybir.dt.float32
BF16 = mybir.dt.bfloat16
MULT = mybir.AluOpType.mult
ADD = mybir.AluOpType.add
MAX = mybir.AluOpType.max
RELU = mybir.ActivationFunctionType.Relu
COPY = mybir.ActivationFunctionType.Copy

# tunables (validated against the deterministic cost-model timeline)
NWARM = 30                # warm-up matmuls (128-col) before real data lands
NFILL1 = 4                # filler matmuls between ep0 and ep1 (dl1 DMA wait)
PSUM_DIRECT_FROM = 59     # eps >= this skip ACT evac; DVE reads PSUM halves
DL_BUFS = 7               # dl tile pool depth

_CACHE = {}


def _build_nc():
    nc = bass.Bass()
    # pre-tiled operands: every DMA descriptor is a full >=512B SBUF row
    xta_d = nc.dram_tensor("xta", (128, KI * 128), BF16, kind="ExternalInput")
    xtb_d = nc.dram_tensor("xtb", (128, KI * 128), BF16, kind="ExternalInput")
    dl_d = nc.dram_tensor("dl", (EP, F, 2 * H), BF16, kind="ExternalInput")
    r0_d = nc.dram_tensor("r0", (128, KI * H), BF16, kind="ExternalInput")
    cols_d = nc.dram_tensor("cols", (BL, E), F32, kind="ExternalInput")
    colse_d = nc.dram_tensor("colse", (E, BL), BF16, kind="ExternalInput")
    d1t_d = nc.dram_tensor("d1t", (E, H), BF16, kind="ExternalInput")
    r1t_d = nc.dram_tensor("r1t", (1, H), BF16, kind="ExternalInput")
    out_d = nc.dram_tensor("out", (BL, 1), F32, kind="ExternalOutput")

    with TileContext(nc) as tc:
        with (
            tc.tile_pool(name="const", bufs=1) as cpool,
            tc.tile_pool(name="acc", bufs=NBT) as apool,
            tc.tile_pool(name="dl", bufs=DL_BUFS) as dpool,
            tc.tile_pool(name="psum", bufs=3, space="PSUM") as ppool,
            tc.tile_pool(name="stage", bufs=8) as spool,
            tc.tile_pool(name="misc", bufs=4) as mpool,
        ):
            # --- resident tiles -------------------------------------------
            # xt_sb free layout: (bt, k, b): lhsT chunk = 128x128 slice
            xt_sb = cpool.tile([128, NBT * KI * 128], BF16, tag="xt")
            r0_sb = cpool.tile([128, KI * H], BF16, tag="r0")
            cols_sb = cpool.tile([128, NBT * E], F32, tag="cols")
            colse_sb = cpool.tile([128, BL], BF16, tag="colse")
            d1t_sb = cpool.tile([128, H], BF16, tag="d1t")
            r1t_sb = cpool.tile([128, H], BF16, tag="r1t")

            dlts = [dpool.tile([128, KI * 2 * H], BF16, tag="dl",
                               name=f"dlt{ep}")
                    for ep in range(EP)]

            def dma_dl(ep, half=None):
                # half=None: whole e-pair tile; half=0/1: k01 / k23 halves
                if half is None:
                    nc.sync.dma_start(
                        dlts[ep][:].rearrange("p (k n) -> p k n", k=KI),
                        dl_d[ep].rearrange("(k p) n -> p k n", p=128),
                    )
                else:
                    sl = slice(half * 256, half * 256 + 256)
                    nc.sync.dma_start(
                        dlts[ep][:, half * KI * H * 2 // 2:
                                 half * KI * H * 2 // 2 + KI * H]
                        .rearrange("p (k n) -> p k n", k=KI // 2),
                        dl_d[ep, sl].rearrange("(k p) n -> p k n", p=128),
                    )

            # --- DMA issue order (critical-path tuned) --------------------
            nc.sync.dma_start(xt_sb[:, : KI * 128], xta_d[:])
            for half in range(2):
                dma_dl(0, half)
            nc.sync.dma_start(xt_sb[:, KI * 128 :], xtb_d[:])
            for half in range(2):
                dma_dl(1, half)
            dma_dl(2)
            nc.sync.dma_start(r0_sb[:, : KI * H // 2], r0_d[:, : KI * H // 2])
            nc.sync.dma_start(r0_sb[:, KI * H // 2 :], r0_d[:, KI * H // 2 :])
            nc.sync.dma_start(
                cols_sb[:].rearrange("p (t n) -> p t n", t=NBT),
                cols_d[:].rearrange("(t p) n -> p t n", p=128),
            )
            for ep in (3, 4, 5):
                dma_dl(ep)
            nc.sync.dma_start(colse_sb[:], colse_d[:])
            nc.sync.dma_start(d1t_sb[:], d1t_d[:])
            nc.sync.dma_start(r1t_sb[:1, :], r1t_d[:])
            for ep in range(6, EP):
                dma_dl(ep)

            ones_sb = cpool.tile([128, 128], BF16, tag="ones")

            # PE warm-up: dummy matmuls on a zeroed scratch tile, issued with
            # no DMA dependency, keep the PE clock-ramp window open while the
            # first data transfers are in flight. The scratch PSUM is never
            # read.
            warm = cpool.tile([128, 128], BF16, tag="warm")
            nc.vector.memset(warm[:], 0.0)
            nc.gpsimd.memset(ones_sb[:1, :], 1.0)
            wps = ppool.tile([128, H], F32, tag="ps_l", name="wps", bufs=5)
            for w in range(NWARM):
                nc.tensor.matmul(
                    wps[:, :128], warm[:], warm[:], start=(w == 0),
                    stop=(w == NWARM - 1),
                )

            def x_lhsT(k, bt):
                # stationary operand: x^T chunk [128 (i), 128 (b)]
                off = bt * KI * 128 + k * 128
                return xt_sb[:, off : off + 128]

            accs = [None, None]
            wts = [None, None]
            res = mpool.tile([128, NBT], F32, tag="res")
            resr = mpool.tile([128, NBT], F32, tag="resr")

            def emit_root(bt):
                # acc[bt] = x @ root0, evacuated straight into acc
                ps = ppool.tile([128, H], F32, tag="ps_l",
                                name=f"ps_r{bt}", bufs=5)
                for k in range(KI):
                    nc.tensor.matmul(
                        ps[:],
                        x_lhsT(k, bt),
                        r0_sb[:, k * H : (k + 1) * H],
                        start=(k == 0),
                        stop=(k == KI - 1),
                    )
                acc = apool.tile([128, H], F32, tag="acc", name=f"acc{bt}")
                nc.scalar.activation(acc[:], ps[:], COPY)
                accs[bt] = acc

            def emit_wts(bt):
                # wt[b,h] = sum_e cols[b,e]*delta1[0,h,e] + root1[h]
                #         = colsE^T @ delta1[0]^T (K=E)  +  ones^T @ root1^T
                psw = ppool.tile([128, H], F32, tag="ps_l",
                                 name=f"ps_w{bt}", bufs=5)
                nc.tensor.matmul(
                    psw[:],
                    colse_sb[:, bt * 128 : (bt + 1) * 128],
                    d1t_sb[:],
                    start=True,
                    stop=False,
                )
                nc.tensor.matmul(
                    psw[:], ones_sb[:1, :], r1t_sb[:1, :], start=False, stop=True
                )
                wt = cpool.tile([128, H], F32, tag=f"wt{bt}")
                nc.scalar.activation(wt[:], psw[:], COPY)
                wts[bt] = wt

            # root / layer-2 weights: emitted here in program order (acc must
            # be initialized before the first STT) — the tile scheduler sinks
            # the matmuls to when r0/colse actually land
            emit_root(0)
            emit_root(1)
            emit_wts(0)
            emit_wts(1)

            # --- stage 1: stream delta, accumulate scaled matmuls ---------
            # PE: ps = x @ [W_{2ep} | W_{2ep+1}] ; ACT: evacuate PSUM->SBUF;
            # DVE: acc = stage_half * colsT[:, e] + acc. bt0 slots of the
            # last eps (and bt0's layer 2) run on the idle Pool engine so the
            # DVE is free for bt1's tail chain. For the last e-pair, bt1 uses
            # per-edge PSUM banks read directly by the DVE (no ACT hop).
            for ep in range(EP):
                last = ep == EP - 1
                direct = ep >= PSUM_DIRECT_FROM
                for bt in range(NBT):
                    if not direct:
                        ps = ppool.tile([128, 2 * H], F32, tag="ps")
                        for k in range(KI):
                            nc.tensor.matmul(
                                ps[:],
                                x_lhsT(k, bt),
                                dlts[ep][:, k * 2 * H : (k + 1) * 2 * H],
                                start=(k == 0),
                                stop=(k == KI - 1),
                            )
                        stage = spool.tile([128, 2 * H], F32, tag="stage")
                        nc.scalar.activation(stage[:], ps[:], COPY)
                        halves = [stage[:, :H], stage[:, H:]]
                    else:
                        # per-edge PSUM banks; DVE reads PSUM directly and
                        # the first edge's STT starts 4 matmuls early
                        halves = []
                        for half in range(2):
                            psh = ppool.tile([128, H], F32, tag="ps_l",
                                             name=f"ps_l{bt}{half}", bufs=5)
                            for k in range(KI):
                                nc.tensor.matmul(
                                    psh[:],
                                    x_lhsT(k, bt),
                                    dlts[ep][:, k * 2 * H + half * H :
                                             k * 2 * H + half * H + H],
                                    start=(k == 0),
                                    stop=(k == KI - 1),
                                )
                            halves.append(psh[:])
                    for half in range(2):
                        e = 2 * ep + half
                        col_ap = cols_sb[:, bt * E + e : bt * E + e + 1]
                        nc.vector.scalar_tensor_tensor(
                            out=accs[bt][:],
                            in0=halves[half],
                            scalar=col_ap,
                            in1=accs[bt][:],
                            op0=MULT,
                            op1=ADD,
                        )
                    if last:
                        # layer 2 for this bt immediately after its last STT
                        junk = mpool.tile([128, H], F32, tag="junk")
                        nc.vector.scalar_tensor_tensor(
                            out=junk[:],
                            in0=accs[bt][:],
                            scalar=0.0,
                            in1=wts[bt][:],
                            op0=MAX,
                            op1=MULT,
                            accum_out=res[:, bt : bt + 1],
                        )
                if ep == 0:
                    # filler keeps the PE ramp hot while dl1 is in flight
                    for _ in range(NFILL1):
                        nc.tensor.matmul(wps[:, :128], warm[:], warm[:],
                                         start=True, stop=True)

            # --- final relu + single combined output DMA ------------------
            # (A prepared SWDGE writeback + trigger would take the 625ns
            # HWDGE gen + 650ns DGE delay off this tail chain, but this
            # walrus build cannot codegen InstTriggerDma: "ISA wrong length".)
            nc.vector.tensor_scalar(resr[:], res[:], 0.0, None, MAX)
            nc.sync.dma_start(
                out_d[:].rearrange("(t p) o -> p (t o)", p=128), resr[:]
            )
    # Surgery on the prepared-writeback protocol:
    # 1. Strip the scheduler's data wait from the PREP: descriptor generation
    #    reads only addresses + the idx tile (Pool-local wait kept); data
    #    ordering vs the relu is enforced on the TRIGGER (step 3).
    # 2. The drain waits on the SWDGE proc sem (DMASW0), which in the
    #    prepared path is never ticked (the completion sem baked into the
    #    descriptors is out_dma) — rewrite those waits to out_dma.
    # 3. Make the TRIGGER wait for the relu via the relu's existing DVE
    #    engine-tick semaphore update (no extra sync commands needed).
    odma_num = out_dma_sem.num
    upd_val = None
    relu_tick = None
    _sem_cum = {}
    for f in nc.m.functions:
        for bb in f.blocks:
            for inst in bb.instructions:
                if inst.sync_info is None:
                    continue
                si = inst.sync_info
                if inst.name == prep_name:
                    si.on_wait = [w for w in si.on_wait
                                  if (w.ant_name or "").startswith("Pool")]
                    for u in si.on_update:
                        if u.id == odma_num:
                            upd_val = u.update_value
                elif inst.name == relu_name:
                    for u in si.on_update:
                        if (u.ant_name or "").startswith("DVE"):
                            cum = _sem_cum.get(u.id, 0) + (u.update_value or 1)
                            relu_tick = (u.id, u.ant_name, cum)
                for u in si.on_update:
                    _sem_cum[u.id] = _sem_cum.get(u.id, 0) + (u.update_value or 1)
    assert upd_val is not None, "out_dma update not found on prep"
    assert relu_tick is not None, "relu DVE tick update not found"
    # The trigger is a fixed-length InstISA: it cannot carry sync waits.
    # Move its waits (plus the new relu-tick wait) onto a NoOp inserted
    # right before it on the Pool engine (the wait-cap legalizer below
    # splits multi-wait NoOps as needed).
    for f in nc.m.functions:
        for bb in f.blocks:
            out = []
            for inst in bb.instructions:
                if inst.name == trig_name:
                    w = bass_rust.SyncWait(
                        sync_type="semaphore", id=relu_tick[0],
                        ant_name=relu_tick[1], wait_mode="sem-ge-imm",
                        wait_value=relu_tick[2],
                    )
                    nop = bass_rust.InstNoOp(
                        name=f"{inst.name}-tw", engine=inst.engine,
                        ins=[], outs=[],
                    )
                    nop.sync_info = bass_rust.SyncInfo(
                        on_wait=[w], on_update=[])
                    out.append(nop)
                out.append(inst)
            bb.instructions = out
    for f in nc.m.functions:
        for bb in f.blocks:
            for inst in bb.instructions:
                si = inst.sync_info
                if si is None:
                    continue
                waits = list(si.on_wait)
                changed = False
                for i, w in enumerate(waits):
                    if (w.ant_name or "").startswith("DMASW"):
                        waits[i] = bass_rust.SyncWait(
                            sync_type="semaphore", id=odma_num,
                            ant_name="out_dma", wait_mode="sem-ge-imm",
                            wait_value=upd_val,
                        )
                        changed = True
                if changed:
                    si.on_wait = waits
    _legalize_wait_counts(nc)
    return nc


def _prep_inputs(x, node_idx, path_mat, root0, root1, delta0, delta1):
    bf16 = ml_dtypes.bfloat16
    x = np.asarray(x, np.float32)
    path_mat = np.asarray(path_mat, np.float32)
    root0 = np.asarray(root0, np.float32)
    root1 = np.asarray(root1, np.float32)
    delta0 = np.asarray(delta0, np.float32)
    delta1 = np.asarray(delta1, np.float32)
    colsT = path_mat.T[np.asarray(node_idx, dtype=np.int64)].astype(np.float32)
    # delta0 (H, F, E) -> (E, F, H) -> pair e's along the free axis
    dt_ = np.ascontiguousarray(delta0.transpose(2, 1, 0))
    dl = np.ascontiguousarray(
        dt_.reshape(EP, 2, F, H).transpose(0, 2, 1, 3)
    ).reshape(EP, F, 2 * H)
    dl16 = dl.astype(bf16)
    xT16 = np.ascontiguousarray(x.T).astype(bf16)  # (F, B)
    # pre-tile r0 to [128 (i%128), (k, h)] so DMA rows are contiguous
    r0t = np.ascontiguousarray(
        root0.reshape(KI, 128, H).transpose(1, 0, 2)
    ).reshape(128, KI * H).astype(bf16)
    colsE16 = np.ascontiguousarray(colsT.T).astype(bf16)  # (E, B)
    d1t = np.ascontiguousarray(delta1[0].T).astype(bf16)  # (E, H)
    r1t = np.ascontiguousarray(root1.T).astype(bf16)  # (1, H)

    in_maps = []
    for c in range(NCORES):
        sl = slice(c * BL, (c + 1) * BL)
        xc = xT16[:, sl]  # (F, BL)
        # pre-tile x^T per bt-half to [128 (i%128), (k, b)]
        xparts = []
        for bt in range(NBT):
            xb = xc[:, bt * 128 : (bt + 1) * 128]  # (F, 128)
            xparts.append(
                np.ascontiguousarray(
                    xb.reshape(KI, 128, 128).transpose(1, 0, 2)
                ).reshape(128, KI * 128)
            )
        in_maps.append(
            {
                "xta": xparts[0],
                "xtb": xparts[1],
                "dl": dl16,
                "r0": r0t,
                "cols": np.ascontiguousarray(colsT[sl]),
                "colse": np.ascontiguousarray(colsE16[:, sl]),
                "d1t": d1t,
                "r1t": r1t,
            }
        )
    return in_maps


def _run(inputs, trace=False, **kw):
    if "nc" not in _CACHE:
        _CACHE["nc"] = _build_nc()
    nc = _CACHE["nc"]
    in_maps = _prep_inputs(**inputs)
    res = run_bass_kernel_spmd(
        nc, in_maps, core_ids=list(range(NCORES)), trace=trace, **kw
    )
    out = np.concatenate([r["out"][:, 0] for r in res.results]).astype(np.float32)
    return out, res


def kernel(**inputs) -> np.ndarray:
    out, _ = _run(inputs)
    return out



# revision 89
# speedup vs baseline: 1.0004x; 1.0004x over previous
"""Trainium2 Bass kernel for nn_DeepNNDendroMatrix.

Math (reference):
    cols = path_mat[:, node_idx]                       # (E, B) in {0,1}
    layer(h, root, delta): relu(h @ root + sum_e cols[e,b] * (h @ W_e))
        where W_e[i,o] = delta[o,i,e]
    out = squeeze(layer2(layer1(x)))

Factorization used here (avoids the (B,in,out) intermediate):
    h1[b,o] = relu( (x@root0)[b,o] + sum_e colsT[b,e] * (x @ W_e)[b,o] )
    out[b]  = relu( sum_h h1[b,h] * wt[b,h] )
        wt[b,h] = root1[h] + sum_e colsT[b,e]*delta1[0,h,e]  (computed up
        front on the PE as colsE^T @ delta1[0]^T + ones @ root1^T)

Distribution: data-parallel over batch. 8 cores x 256 samples. Each core
streams the full (rearranged, bf16) delta0 once from HBM (33.5 MB), keeps
x^T resident in SBUF as the matmul stationary operand, accumulates the
per-edge scaled matmul outputs on the vector engine with fused
scalar_tensor_tensor (acc = psum_e * colsT[:,e] + acc).

The main PE stream (64 e-pairs x 2 batch tiles x 4 K-chunks x 512-col
matmuls) is the hard floor at ~109.2us; the schedule is tuned so the PE
runs it back-to-back:
  - all DMA sources are host-pre-tiled so every descriptor is a full
    >=512B row (no read-modify-write penalty), and the issue order is
    tuned against the serialized HWDGE descriptor-gen (625ns each) and
    DMA-engine (360B/ns) stages so each e-pair lands just before the PE
    needs it;
  - NWARM warm-up matmuls on a zeroed tile (plus NFILL1 fillers after
    ep0) keep the PE clock ramp hot until real data arrives;
  - x@root0 is evacuated straight into the accumulator (no separate
    memset/add) and root/wts matmuls are emitted early so the scheduler
    sinks them into DMA-wait gaps;
  - for the tail, the last e-pairs skip the ACT evacuation (DVE reads
    the PSUM halves directly) and the final e-pair accumulates each edge
    in its own PSUM bank so the first STT starts 4 matmuls early; the
    final relu runs on the DVE and a single combined DMA writes both
    batch tiles' outputs.
"""

import numpy as np
import ml_dtypes

import concourse.bass as bass
import concourse.mybir as mybir
from concourse.tile import TileContext
from concourse.bass_utils import run_bass_kernel_spmd

# ---------------------------------------------------------------------------
# Workaround: this walrus build allows only ONE sync wait per CTRL (Drain)
# instruction; TileContext's tail drain aggregates one wait per live
# semaphore onto a single Drain. Split them across multiple Drains.
import bass_rust
import concourse.tile as _tile_mod
from concourse.vector_clock import ScopedClock as _ScopedClock

_MAX_WAITS_PER_INST = 1


def _split_drain_and_barrier(self, tick_clock, wait_clock):
    nc = self.nc
    drain_inst = nc.sync.drain()
    wait_clock.add_sem_waits(
        drain_inst.ins, _ScopedClock({None: tick_clock.global_clock})
    )
    si = drain_inst.ins.sync_info
    waits = list(si.on_wait) if si is not None else []
    # DMA-queue sems complete last (out-DMA + 900ns prop): drain them after
    # the engine sems so those drains retire while the DMA is in flight
    waits.sort(key=lambda w: (w.ant_name or "").startswith(("DMAHW", "DMASW")))
    if len(waits) > _MAX_WAITS_PER_INST:
        si.on_wait = waits[:_MAX_WAITS_PER_INST]
        rest = waits[_MAX_WAITS_PER_INST:]
        for i in range(0, len(rest), _MAX_WAITS_PER_INST):
            extra = nc.sync.drain()
            chunk = rest[i : i + _MAX_WAITS_PER_INST]
            esi = extra.ins.sync_info
            if esi is None:
                extra.ins.sync_info = bass_rust.SyncInfo(on_wait=chunk, on_update=[])
            else:
                esi.on_wait = list(esi.on_wait) + chunk
    nc.all_engine_barrier()
    assert self.sems is not None
    popped = nc._tile_sem_poison_stack.pop()
    assert popped is self._sem_poison
    nc.clear_and_free_semaphores(list(self.sems.allocated().values()))
    nc.all_engine_barrier()


_tile_mod.TileContext._drain_and_barrier = _split_drain_and_barrier


_COALESCE_OK = {"Ldweights", "NoOp", "TensorCopy", "Memset", "TensorScalarPtr",
                "Matmult", "Activation", "TensorScalar"}


import os as _os2

_WAIT_CAP_DEFAULT = int(_os2.environ.get("KW_WAIT_CAP", "1"))


def _legalize_wait_counts(nc, max_waits=None):
    """Split any instruction carrying more than `max_waits` sync waits.

    Moving a wait onto an earlier instruction of the same engine is always
    safe (the engine just blocks earlier), so first try to coalesce excess
    waits onto the immediately-preceding same-engine instruction if it has
    spare wait slots; otherwise insert a NoOp carrying the wait."""
    if max_waits is None:
        max_waits = _WAIT_CAP_DEFAULT
    n_nops = 0
    for f in nc.m.functions:
        for bb in f.blocks:
            out = []
            for inst in bb.instructions:
                si = inst.sync_info
                waits = list(si.on_wait) if si is not None else []
                if len(waits) > max_waits:
                    si.on_wait = waits[:max_waits]
                    rest = waits[max_waits:]
                    # try to place excess on the immediately-preceding
                    # same-engine instruction (moving a wait earlier on the
                    # same engine is always safe, as long as that instruction
                    # does not itself update the awaited semaphore)
                    if out:
                        prev = out[-1]
                        if prev.engine == inst.engine and prev.opcode in _COALESCE_OK:
                            psi = prev.sync_info
                            pw = list(psi.on_wait) if psi is not None else []
                            upd_ids = {
                                u.id
                                for u in (psi.on_update if psi is not None else [])
                            }
                            while (
                                rest
                                and len(pw) < max_waits
                                and rest[0].id not in upd_ids
                            ):
                                pw.append(rest.pop(0))
                            if pw:
                                if psi is None:
                                    prev.sync_info = bass_rust.SyncInfo(
                                        on_wait=pw, on_update=[]
                                    )
                                else:
                                    psi.on_wait = pw
                    for i in range(0, len(rest), max_waits):
                        nop = bass_rust.InstNoOp(
                            name=f"{inst.name}-ws{i}", engine=inst.engine,
                            ins=[], outs=[],
                        )
                        nop.sync_info = bass_rust.SyncInfo(
                            on_wait=rest[i : i + max_waits], on_update=[]
                        )
                        out.append(nop)
                        n_nops += 1
                out.append(inst)
            bb.instructions = out
    return n_nops
# ---------------------------------------------------------------------------

# ---------------------------------------------------------------------------
# Persistent NEFF cache: walrus compilation of this kernel takes minutes and
# bass2jax recompiles per process. Cache the compiled NEFF on disk keyed by
# the BIR sha256 so repeat processes skip the compile.
import hashlib as _hashlib
import os as _os
import shutil as _shutil

import concourse.bass2jax as _bass2jax
import concourse.bass_utils as _bass_utils_mod

_NEFF_CACHE_DIR = _os.path.expanduser("~/.cache/bass_neff")
_orig_compile_bir_kernel = _bass_utils_mod.compile_bir_kernel


def _bir_cache_key(raw: bytes) -> str:
    """sha256 of the BIR with all debug info stripped: ant_debug blobs embed
    full tracebacks (including the CALLER's file/line), which vary with the
    directory and script kernel.py is invoked from."""
    import orjson

    d = orjson.loads(raw)
    d.pop("debug_table", None)

    def scrub(o):
        if isinstance(o, dict):
            o.pop("ant_debug", None)
            o.pop("debug", None)
            for v in o.values():
                scrub(v)
        elif isinstance(o, list):
            for v in o:
                scrub(v)

    scrub(d)
    return _hashlib.sha256(orjson.dumps(d)).hexdigest()


def _source_cache_key():
    """Fallback key: hash of this file's exact contents (+ env knobs that
    affect codegen). The builder is deterministic given the source, and a
    NEFF compiled in one calling context is valid in any other (tensor
    bindings are positional), so this key is safe when the BIR-byte key
    misses due to residual context-dependent debug noise."""
    with open(__file__, "rb") as f:
        src = f.read()
    return _hashlib.sha256(
        src + str(_WAIT_CAP_DEFAULT).encode()
    ).hexdigest()


def _cached_compile_bir_kernel(bir_json, tmpdir, neff_name="file.neff"):
    try:
        raw = bir_json if isinstance(bir_json, bytes) else bir_json.encode()
        keys = [_bir_cache_key(raw), "src" + _source_cache_key()]
        cpaths = [
            _os.path.join(_NEFF_CACHE_DIR, f"{k}_{neff_name}") for k in keys
        ]
        for cpath in cpaths:
            if _os.path.exists(cpath):
                dst = _os.path.join(tmpdir, "sg00")
                _os.makedirs(dst, exist_ok=True)
                dst_neff = _os.path.join(dst, neff_name)
                _shutil.copy(cpath, dst_neff)
                return dst_neff
    except Exception:
        return _orig_compile_bir_kernel(bir_json, tmpdir, neff_name)
    neff_path = _orig_compile_bir_kernel(bir_json, tmpdir, neff_name)
    try:
        _os.makedirs(_NEFF_CACHE_DIR, exist_ok=True)
        for cpath in cpaths:
            tmp = cpath + ".tmp"
            _shutil.copy(neff_path, tmp)
            _os.replace(tmp, cpath)
    except Exception:
        pass
    return neff_path


_bass2jax.compile_bir_kernel = _cached_compile_bir_kernel
_bass_utils_mod.compile_bir_kernel = _cached_compile_bir_kernel
# ---------------------------------------------------------------------------

NCORES = 8
B, F, H, O, E, N_NODES = 2048, 512, 256, 1, 128, 4096
BL = B // NCORES          # samples per core = 256
NBT = BL // 128           # b-tiles per core = 2
EP = E // 2               # e-pairs = 64
KI = F // 128             # contraction chunks over input features = 4

F32 = mybir.dt.float32
BF16 = mybir.dt.bfloat16
MULT = mybir.AluOpType.mult
ADD = mybir.AluOpType.add
MAX = mybir.AluOpType.max
RELU = mybir.ActivationFunctionType.Relu
COPY = mybir.ActivationFunctionType.Copy

# tunables (validated against the deterministic cost-model timeline)
NWARM = 30                # warm-up matmuls (128-col) before real data lands
NFILL1 = 4                # filler matmuls between ep0 and ep1 (dl1 DMA wait)
PSUM_DIRECT_FROM = 59     # eps >= this skip ACT evac; DVE reads PSUM halves
DL_BUFS = 7               # dl tile pool depth

_CACHE = {}


def _build_nc():
    nc = bass.Bass()
    # pre-tiled operands: every DMA descriptor is a full >=512B SBUF row
    hd0_d = nc.dram_tensor("hd0", (128, KI * 128 + KI * H), BF16,
                           kind="ExternalInput")
    xtb_d = nc.dram_tensor("xtb", (128, KI * 128), BF16, kind="ExternalInput")
    dl_d = nc.dram_tensor("dl", (EP, F, 2 * H), BF16, kind="ExternalInput")
    r0_d = nc.dram_tensor("r0", (128, KI * H), BF16, kind="ExternalInput")
    cols_d = nc.dram_tensor("cols", (BL, E), F32, kind="ExternalInput")
    colse_d = nc.dram_tensor("colse", (E, BL), BF16, kind="ExternalInput")
    d1t_d = nc.dram_tensor("d1t", (E, H), BF16, kind="ExternalInput")
    r1t_d = nc.dram_tensor("r1t", (1, H), BF16, kind="ExternalInput")
    out_d = nc.dram_tensor("out", (BL, 1), F32, kind="ExternalOutput")

    with TileContext(nc) as tc:
        with (
            tc.tile_pool(name="const", bufs=1) as cpool,
            tc.tile_pool(name="acc", bufs=NBT) as apool,
            tc.tile_pool(name="dl", bufs=DL_BUFS) as dpool,
            tc.tile_pool(name="psum", bufs=3, space="PSUM") as ppool,
            tc.tile_pool(name="stage", bufs=8) as spool,
            tc.tile_pool(name="misc", bufs=4) as mpool,
        ):
            # --- resident tiles -------------------------------------------
            # head: [xta (k,b) | dl0 (k,n)] fused so one DMA carries the
            # first x half plus dl0-k01 (one HWDGE gen on the critical head)
            head = cpool.tile([128, KI * 128 + KI * 2 * H], BF16, tag="head")
            xt_sb = cpool.tile([128, KI * 128], BF16, tag="xt")
            r0_sb = cpool.tile([128, KI * H], BF16, tag="r0")
            cols_sb = cpool.tile([128, NBT * E], F32, tag="cols")
            colse_sb = cpool.tile([128, BL], BF16, tag="colse")
            d1t_sb = cpool.tile([128, H], BF16, tag="d1t")
            r1t_sb = cpool.tile([128, H], BF16, tag="r1t")

            dlts = [dpool.tile([128, KI * 2 * H], BF16, tag="dl",
                               name=f"dlt{ep}")
                    for ep in range(EP)]

            def dl_rhs(ep, lo, hi):
                if ep == 0:
                    off = KI * 128
                    return head[:, off + lo : off + hi]
                return dlts[ep][:, lo:hi]

            def dma_dl(ep, half=None):
                # half=None: whole e-pair tile; half=0/1: k01 / k23 halves
                if ep == 0:
                    # k23 half into the fused head tile (k01 rides with hd0)
                    assert half == 1
                    nc.sync.dma_start(
                        head[:, KI * 128 + KI * H :]
                        .rearrange("p (k n) -> p k n", k=KI // 2),
                        dl_d[0, 256:].rearrange("(k p) n -> p k n", p=128),
                    )
                    return
                if half is None:
                    nc.sync.dma_start(
                        dlts[ep][:].rearrange("p (k n) -> p k n", k=KI),
                        dl_d[ep].rearrange("(k p) n -> p k n", p=128),
                    )
                else:
                    sl = slice(half * 256, half * 256 + 256)
                    nc.sync.dma_start(
                        dlts[ep][:, half * KI * H * 2 // 2:
                                 half * KI * H * 2 // 2 + KI * H]
                        .rearrange("p (k n) -> p k n", k=KI // 2),
                        dl_d[ep, sl].rearrange("(k p) n -> p k n", p=128),
                    )

            # --- DMA issue order (critical-path tuned) --------------------
            nc.sync.dma_start(head[:, : KI * 128 + KI * H], hd0_d[:])
            dma_dl(0, 1)
            nc.sync.dma_start(xt_sb[:], xtb_d[:])
            for half in range(2):
                dma_dl(1, half)
            dma_dl(2)
            nc.sync.dma_start(r0_sb[:, : KI * H // 2], r0_d[:, : KI * H // 2])
            nc.sync.dma_start(r0_sb[:, KI * H // 2 :], r0_d[:, KI * H // 2 :])
            nc.sync.dma_start(
                cols_sb[:].rearrange("p (t n) -> p t n", t=NBT),
                cols_d[:].rearrange("(t p) n -> p t n", p=128),
            )
            for ep in (3, 4, 5):
                dma_dl(ep)
            nc.sync.dma_start(colse_sb[:], colse_d[:])
            nc.sync.dma_start(d1t_sb[:], d1t_d[:])
            nc.sync.dma_start(r1t_sb[:1, :], r1t_d[:])
            for ep in range(6, EP):
                dma_dl(ep)

            ones_sb = cpool.tile([128, 128], BF16, tag="ones")

            # PE warm-up: dummy matmuls on a zeroed scratch tile, issued with
            # no DMA dependency, keep the PE clock-ramp window open while the
            # first data transfers are in flight. The scratch PSUM is never
            # read.
            warm = cpool.tile([128, 128], BF16, tag="warm")
            nc.vector.memset(warm[:], 0.0)
            nc.gpsimd.memset(ones_sb[:1, :], 1.0)
            wps = ppool.tile([128, H], F32, tag="ps_l", name="wps", bufs=5)
            for w in range(NWARM):
                nc.tensor.matmul(
                    wps[:, :128], warm[:], warm[:], start=(w == 0),
                    stop=(w == NWARM - 1),
                )

            def x_lhsT(k, bt):
                # stationary operand: x^T chunk [128 (i), 128 (b)]
                base = head if bt == 0 else xt_sb
                return base[:, k * 128 : k * 128 + 128]

            accs = [None, None]
            wts = [None, None]
            res = mpool.tile([128, NBT], F32, tag="res")
            resr = mpool.tile([128, NBT], F32, tag="resr")

            def emit_root(bt):
                # acc[bt] = x @ root0, evacuated straight into acc
                ps = ppool.tile([128, H], F32, tag="ps_l",
                                name=f"ps_r{bt}", bufs=5)
                for k in range(KI):
                    nc.tensor.matmul(
                        ps[:],
                        x_lhsT(k, bt),
                        r0_sb[:, k * H : (k + 1) * H],
                        start=(k == 0),
                        stop=(k == KI - 1),
                    )
                acc = apool.tile([128, H], F32, tag="acc", name=f"acc{bt}")
                nc.scalar.activation(acc[:], ps[:], COPY)
                accs[bt] = acc

            def emit_wts(bt):
                # wt[b,h] = sum_e cols[b,e]*delta1[0,h,e] + root1[h]
                #         = colsE^T @ delta1[0]^T (K=E)  +  ones^T @ root1^T
                psw = ppool.tile([128, H], F32, tag="ps_l",
                                 name=f"ps_w{bt}", bufs=5)
                nc.tensor.matmul(
                    psw[:],
                    colse_sb[:, bt * 128 : (bt + 1) * 128],
                    d1t_sb[:],
                    start=True,
                    stop=False,
                )
                nc.tensor.matmul(
                    psw[:], ones_sb[:1, :], r1t_sb[:1, :], start=False, stop=True
                )
                wt = cpool.tile([128, H], F32, tag=f"wt{bt}")
                nc.scalar.activation(wt[:], psw[:], COPY)
                wts[bt] = wt

            # root / layer-2 weights: emitted here in program order (acc must
            # be initialized before the first STT) — the tile scheduler sinks
            # the matmuls to when r0/colse actually land
            emit_root(0)
            emit_root(1)
            emit_wts(0)
            emit_wts(1)

            # --- stage 1: stream delta, accumulate scaled matmuls ---------
            # PE: ps = x @ [W_{2ep} | W_{2ep+1}] ; ACT: evacuate PSUM->SBUF;
            # DVE: acc = stage_half * colsT[:, e] + acc. bt0 slots of the
            # last eps (and bt0's layer 2) run on the idle Pool engine so the
            # DVE is free for bt1's tail chain. For the last e-pair, bt1 uses
            # per-edge PSUM banks read directly by the DVE (no ACT hop).
            for ep in range(EP):
                last = ep == EP - 1
                direct = ep >= PSUM_DIRECT_FROM
                for bt in range(NBT):
                    if not direct:
                        ps = ppool.tile([128, 2 * H], F32, tag="ps")
                        for k in range(KI):
                            nc.tensor.matmul(
                                ps[:],
                                x_lhsT(k, bt),
                                dl_rhs(ep, k * 2 * H, (k + 1) * 2 * H),
                                start=(k == 0),
                                stop=(k == KI - 1),
                            )
                        stage = spool.tile([128, 2 * H], F32, tag="stage")
                        nc.scalar.activation(stage[:], ps[:], COPY)
                        halves = [stage[:, :H], stage[:, H:]]
                    else:
                        # per-edge PSUM banks; DVE reads PSUM directly and
                        # the first edge's STT starts 4 matmuls early
                        halves = []
                        for half in range(2):
                            psh = ppool.tile([128, H], F32, tag="ps_l",
                                             name=f"ps_l{bt}{half}", bufs=5)
                            for k in range(KI):
                                nc.tensor.matmul(
                                    psh[:],
                                    x_lhsT(k, bt),
                                    dl_rhs(ep, k * 2 * H + half * H,
                                           k * 2 * H + half * H + H),
                                    start=(k == 0),
                                    stop=(k == KI - 1),
                                )
                            halves.append(psh[:])
                    for half in range(2):
                        e = 2 * ep + half
                        col_ap = cols_sb[:, bt * E + e : bt * E + e + 1]
                        nc.vector.scalar_tensor_tensor(
                            out=accs[bt][:],
                            in0=halves[half],
                            scalar=col_ap,
                            in1=accs[bt][:],
                            op0=MULT,
                            op1=ADD,
                        )
                    if last:
                        # layer 2 for this bt immediately after its last STT
                        junk = mpool.tile([128, H], F32, tag="junk")
                        nc.vector.scalar_tensor_tensor(
                            out=junk[:],
                            in0=accs[bt][:],
                            scalar=0.0,
                            in1=wts[bt][:],
                            op0=MAX,
                            op1=MULT,
                            accum_out=res[:, bt : bt + 1],
                        )
                if ep == 0:
                    # filler keeps the PE ramp hot while dl1 is in flight
                    for _ in range(NFILL1):
                        nc.tensor.matmul(wps[:, :128], warm[:], warm[:],
                                         start=True, stop=True)

            # --- final relu + single combined output DMA ------------------
            # (A prepared SWDGE writeback + trigger would take the 625ns
            # HWDGE gen + 650ns DGE delay off this tail chain, but this
            # walrus build cannot codegen InstTriggerDma: "ISA wrong length".)
            nc.vector.tensor_scalar(resr[:], res[:], 0.0, None, MAX)
            nc.sync.dma_start(
                out_d[:].rearrange("(t p) o -> p (t o)", p=128), resr[:]
            )
    # Surgery on the prepared-writeback protocol:
    # 1. Strip the scheduler's data wait from the PREP: descriptor generation
    #    reads only addresses + the idx tile (Pool-local wait kept); data
    #    ordering vs the relu is enforced on the TRIGGER (step 3).
    # 2. The drain waits on the SWDGE proc sem (DMASW0), which in the
    #    prepared path is never ticked (the completion sem baked into the
    #    descriptors is out_dma) — rewrite those waits to out_dma.
    # 3. Make the TRIGGER wait for the relu via the relu's existing DVE
    #    engine-tick semaphore update (no extra sync commands needed).
    odma_num = out_dma_sem.num
    upd_val = None
    relu_tick = None
    _sem_cum = {}
    for f in nc.m.functions:
        for bb in f.blocks:
            for inst in bb.instructions:
                if inst.sync_info is None:
                    continue
                si = inst.sync_info
                if inst.name == prep_name:
                    si.on_wait = [w for w in si.on_wait
                                  if (w.ant_name or "").startswith("Pool")]
                    for u in si.on_update:
                        if u.id == odma_num:
                            upd_val = u.update_value
                elif inst.name == relu_name:
                    for u in si.on_update:
                        if (u.ant_name or "").startswith("DVE"):
                            cum = _sem_cum.get(u.id, 0) + (u.update_value or 1)
                            relu_tick = (u.id, u.ant_name, cum)
                for u in si.on_update:
                    _sem_cum[u.id] = _sem_cum.get(u.id, 0) + (u.update_value or 1)
    assert upd_val is not None, "out_dma update not found on prep"
    assert relu_tick is not None, "relu DVE tick update not found"
    # The trigger is a fixed-length InstISA: it cannot carry sync waits.
    # Move its waits (plus the new relu-tick wait) onto a NoOp inserted
    # right before it on the Pool engine (the wait-cap legalizer below
    # splits multi-wait NoOps as needed).
    for f in nc.m.functions:
        for bb in f.blocks:
            out = []
            for inst in bb.instructions:
                if inst.name == trig_name:
                    w = bass_rust.SyncWait(
                        sync_type="semaphore", id=relu_tick[0],
                        ant_name=relu_tick[1], wait_mode="sem-ge-imm",
                        wait_value=relu_tick[2],
                    )
                    nop = bass_rust.InstNoOp(
                        name=f"{inst.name}-tw", engine=inst.engine,
                        ins=[], outs=[],
                    )
                    nop.sync_info = bass_rust.SyncInfo(
                        on_wait=[w], on_update=[])
                    out.append(nop)
                out.append(inst)
            bb.instructions = out
    for f in nc.m.functions:
        for bb in f.blocks:
            for inst in bb.instructions:
                si = inst.sync_info
                if si is None:
                    continue
                waits = list(si.on_wait)
                changed = False
                for i, w in enumerate(waits):
                    if (w.ant_name or "").startswith("DMASW"):
                        waits[i] = bass_rust.SyncWait(
                            sync_type="semaphore", id=odma_num,
                            ant_name="out_dma", wait_mode="sem-ge-imm",
                            wait_value=upd_val,
                        )
                        changed = True
                if changed:
                    si.on_wait = waits
    _legalize_wait_counts(nc)
    return nc


def _prep_inputs(x, node_idx, path_mat, root0, root1, delta0, delta1):
    bf16 = ml_dtypes.bfloat16
    x = np.asarray(x, np.float32)
    path_mat = np.asarray(path_mat, np.float32)
    root0 = np.asarray(root0, np.float32)
    root1 = np.asarray(root1, np.float32)
    delta0 = np.asarray(delta0, np.float32)
    delta1 = np.asarray(delta1, np.float32)
    colsT = path_mat.T[np.asarray(node_idx, dtype=np.int64)].astype(np.float32)
    # delta0 (H, F, E) -> (E, F, H) -> pair e's along the free axis
    dt_ = np.ascontiguousarray(delta0.transpose(2, 1, 0))
    dl = np.ascontiguousarray(
        dt_.reshape(EP, 2, F, H).transpose(0, 2, 1, 3)
    ).reshape(EP, F, 2 * H)
    dl16 = dl.astype(bf16)
    xT16 = np.ascontiguousarray(x.T).astype(bf16)  # (F, B)
    # pre-tile r0 to [128 (i%128), (k, h)] so DMA rows are contiguous
    r0t = np.ascontiguousarray(
        root0.reshape(KI, 128, H).transpose(1, 0, 2)
    ).reshape(128, KI * H).astype(bf16)
    colsE16 = np.ascontiguousarray(colsT.T).astype(bf16)  # (E, B)
    d1t = np.ascontiguousarray(delta1[0].T).astype(bf16)  # (E, H)
    r1t = np.ascontiguousarray(root1.T).astype(bf16)  # (1, H)

    in_maps = []
    for c in range(NCORES):
        sl = slice(c * BL, (c + 1) * BL)
        xc = xT16[:, sl]  # (F, BL)
        # pre-tile x^T per bt-half to [128 (i%128), (k, b)]
        xparts = []
        for bt in range(NBT):
            xb = xc[:, bt * 128 : (bt + 1) * 128]  # (F, 128)
            xparts.append(
                np.ascontiguousarray(
                    xb.reshape(KI, 128, 128).transpose(1, 0, 2)
                ).reshape(128, KI * 128)
            )
        dl0k01 = np.ascontiguousarray(
            np.asarray(dl16[0][:256]).reshape(2, 128, 2 * H)
            .transpose(1, 0, 2)
        ).reshape(128, KI * H)
        hd0 = np.concatenate([np.asarray(xparts[0]), dl0k01], axis=1)
        in_maps.append(
            {
                "hd0": np.ascontiguousarray(hd0),
                "xtb": xparts[1],
                "dl": dl16,
                "r0": r0t,
                "cols": np.ascontiguousarray(colsT[sl]),
                "colse": np.ascontiguousarray(colsE16[:, sl]),
                "d1t": d1t,
                "r1t": r1t,
            }
        )
    return in_maps


def _run(inputs, trace=False, **kw):
    if "nc" not in _CACHE:
        _CACHE["nc"] = _build_nc()
    nc = _CACHE["nc"]
    in_maps = _prep_inputs(**inputs)
    res = run_bass_kernel_spmd(
        nc, in_maps, core_ids=list(range(NCORES)), trace=trace, **kw
    )
    out = np.concatenate([r["out"][:, 0] for r in res.results]).astype(np.float32)
    return out, res


def kernel(**inputs) -> np.ndarray:
    out, _ = _run(inputs)
    return out



# revision 92
# speedup vs baseline: 1.0033x; 1.0029x over previous
"""Trainium2 Bass kernel for nn_DeepNNDendroMatrix.

Math (reference):
    cols = path_mat[:, node_idx]                       # (E, B) in {0,1}
    layer(h, root, delta): relu(h @ root + sum_e cols[e,b] * (h @ W_e))
        where W_e[i,o] = delta[o,i,e]
    out = squeeze(layer2(layer1(x)))

Factorization used here (avoids the (B,in,out) intermediate):
    h1[b,o] = relu( (x@root0)[b,o] + sum_e colsT[b,e] * (x @ W_e)[b,o] )
    out[b]  = relu( sum_h h1[b,h] * wt[b,h] )
        wt[b,h] = root1[h] + sum_e colsT[b,e]*delta1[0,h,e]  (computed up
        front on the PE as colsE^T @ delta1[0]^T + ones @ root1^T)

Distribution: data-parallel over batch. 8 cores x 256 samples. Each core
streams the full (rearranged, bf16) delta0 once from HBM (33.5 MB), keeps
x^T resident in SBUF as the matmul stationary operand, accumulates the
per-edge scaled matmul outputs on the vector engine with fused
scalar_tensor_tensor (acc = psum_e * colsT[:,e] + acc).

The main PE stream (64 e-pairs x 2 batch tiles x 4 K-chunks x 512-col
matmuls) is the hard floor at ~109.2us; the schedule is tuned so the PE
runs it back-to-back:
  - all DMA sources are host-pre-tiled so every descriptor is a full
    >=512B row (no read-modify-write penalty), and the issue order is
    tuned against the serialized HWDGE descriptor-gen (625ns each) and
    DMA-engine (360B/ns) stages so each e-pair lands just before the PE
    needs it;
  - NWARM warm-up matmuls on a zeroed tile (plus NFILL1 fillers after
    ep0) keep the PE clock ramp hot until real data arrives;
  - x@root0 is evacuated straight into the accumulator (no separate
    memset/add) and root/wts matmuls are emitted early so the scheduler
    sinks them into DMA-wait gaps;
  - for the tail, the last e-pairs skip the ACT evacuation (DVE reads
    the PSUM halves directly) and the final e-pair accumulates each edge
    in its own PSUM bank so the first STT starts 4 matmuls early; the
    final relu runs on the DVE and a single combined DMA writes both
    batch tiles' outputs.
"""

import numpy as np
import ml_dtypes

import concourse.bass as bass
import concourse.mybir as mybir
from concourse.tile import TileContext
from concourse.bass_utils import run_bass_kernel_spmd

# ---------------------------------------------------------------------------
# Workaround: this walrus build allows only ONE sync wait per CTRL (Drain)
# instruction; TileContext's tail drain aggregates one wait per live
# semaphore onto a single Drain. Split them across multiple Drains.
import bass_rust
import concourse.tile as _tile_mod
from concourse.vector_clock import ScopedClock as _ScopedClock

_MAX_WAITS_PER_INST = 1


def _split_drain_and_barrier(self, tick_clock, wait_clock):
    nc = self.nc
    drain_inst = nc.sync.drain()
    wait_clock.add_sem_waits(
        drain_inst.ins, _ScopedClock({None: tick_clock.global_clock})
    )
    si = drain_inst.ins.sync_info
    waits = list(si.on_wait) if si is not None else []
    # DMA-queue sems complete last (out-DMA + 900ns prop): drain them after
    # the engine sems so those drains retire while the DMA is in flight
    waits.sort(key=lambda w: (w.ant_name or "").startswith(("DMAHW", "DMASW")))
    if len(waits) > _MAX_WAITS_PER_INST:
        si.on_wait = waits[:_MAX_WAITS_PER_INST]
        rest = waits[_MAX_WAITS_PER_INST:]
        for i in range(0, len(rest), _MAX_WAITS_PER_INST):
            extra = nc.sync.drain()
            chunk = rest[i : i + _MAX_WAITS_PER_INST]
            esi = extra.ins.sync_info
            if esi is None:
                extra.ins.sync_info = bass_rust.SyncInfo(on_wait=chunk, on_update=[])
            else:
                esi.on_wait = list(esi.on_wait) + chunk
    nc.all_engine_barrier()
    assert self.sems is not None
    popped = nc._tile_sem_poison_stack.pop()
    assert popped is self._sem_poison
    nc.clear_and_free_semaphores(list(self.sems.allocated().values()))
    nc.all_engine_barrier()


_tile_mod.TileContext._drain_and_barrier = _split_drain_and_barrier


_COALESCE_OK = {"Ldweights", "NoOp", "TensorCopy", "Memset", "TensorScalarPtr",
                "Matmult", "Activation", "TensorScalar"}


import os as _os2

_WAIT_CAP_DEFAULT = int(_os2.environ.get("KW_WAIT_CAP", "1"))


def _legalize_wait_counts(nc, max_waits=None):
    """Split any instruction carrying more than `max_waits` sync waits.

    Moving a wait onto an earlier instruction of the same engine is always
    safe (the engine just blocks earlier), so first try to coalesce excess
    waits onto the immediately-preceding same-engine instruction if it has
    spare wait slots; otherwise insert a NoOp carrying the wait."""
    if max_waits is None:
        max_waits = _WAIT_CAP_DEFAULT
    n_nops = 0
    for f in nc.m.functions:
        for bb in f.blocks:
            out = []
            for inst in bb.instructions:
                si = inst.sync_info
                waits = list(si.on_wait) if si is not None else []
                if len(waits) > max_waits:
                    si.on_wait = waits[:max_waits]
                    rest = waits[max_waits:]
                    # try to place excess on the immediately-preceding
                    # same-engine instruction (moving a wait earlier on the
                    # same engine is always safe, as long as that instruction
                    # does not itself update the awaited semaphore)
                    if out:
                        prev = out[-1]
                        if prev.engine == inst.engine and prev.opcode in _COALESCE_OK:
                            psi = prev.sync_info
                            pw = list(psi.on_wait) if psi is not None else []
                            upd_ids = {
                                u.id
                                for u in (psi.on_update if psi is not None else [])
                            }
                            while (
                                rest
                                and len(pw) < max_waits
                                and rest[0].id not in upd_ids
                            ):
                                pw.append(rest.pop(0))
                            if pw:
                                if psi is None:
                                    prev.sync_info = bass_rust.SyncInfo(
                                        on_wait=pw, on_update=[]
                                    )
                                else:
                                    psi.on_wait = pw
                    for i in range(0, len(rest), max_waits):
                        nop = bass_rust.InstNoOp(
                            name=f"{inst.name}-ws{i}", engine=inst.engine,
                            ins=[], outs=[],
                        )
                        nop.sync_info = bass_rust.SyncInfo(
                            on_wait=rest[i : i + max_waits], on_update=[]
                        )
                        out.append(nop)
                        n_nops += 1
                out.append(inst)
            bb.instructions = out
    return n_nops
# ---------------------------------------------------------------------------

# ---------------------------------------------------------------------------
# Persistent NEFF cache: walrus compilation of this kernel takes minutes and
# bass2jax recompiles per process. Cache the compiled NEFF on disk keyed by
# the BIR sha256 so repeat processes skip the compile.
import hashlib as _hashlib
import os as _os
import shutil as _shutil

import concourse.bass2jax as _bass2jax
import concourse.bass_utils as _bass_utils_mod

_NEFF_CACHE_DIR = _os.path.expanduser("~/.cache/bass_neff")
_orig_compile_bir_kernel = _bass_utils_mod.compile_bir_kernel


def _bir_cache_key(raw: bytes) -> str:
    """sha256 of the BIR with all debug info stripped: ant_debug blobs embed
    full tracebacks (including the CALLER's file/line), which vary with the
    directory and script kernel.py is invoked from."""
    import orjson

    d = orjson.loads(raw)
    d.pop("debug_table", None)

    def scrub(o):
        if isinstance(o, dict):
            o.pop("ant_debug", None)
            o.pop("debug", None)
            for v in o.values():
                scrub(v)
        elif isinstance(o, list):
            for v in o:
                scrub(v)

    scrub(d)
    return _hashlib.sha256(orjson.dumps(d)).hexdigest()


def _source_cache_key():
    """Fallback key: hash of this file's exact contents (+ env knobs that
    affect codegen). The builder is deterministic given the source, and a
    NEFF compiled in one calling context is valid in any other (tensor
    bindings are positional), so this key is safe when the BIR-byte key
    misses due to residual context-dependent debug noise."""
    with open(__file__, "rb") as f:
        src = f.read()
    return _hashlib.sha256(
        src + str(_WAIT_CAP_DEFAULT).encode()
    ).hexdigest()


def _cached_compile_bir_kernel(bir_json, tmpdir, neff_name="file.neff"):
    try:
        raw = bir_json if isinstance(bir_json, bytes) else bir_json.encode()
        keys = [_bir_cache_key(raw), "src" + _source_cache_key()]
        cpaths = [
            _os.path.join(_NEFF_CACHE_DIR, f"{k}_{neff_name}") for k in keys
        ]
        for cpath in cpaths:
            if _os.path.exists(cpath):
                dst = _os.path.join(tmpdir, "sg00")
                _os.makedirs(dst, exist_ok=True)
                dst_neff = _os.path.join(dst, neff_name)
                _shutil.copy(cpath, dst_neff)
                return dst_neff
    except Exception:
        return _orig_compile_bir_kernel(bir_json, tmpdir, neff_name)
    neff_path = _orig_compile_bir_kernel(bir_json, tmpdir, neff_name)
    try:
        _os.makedirs(_NEFF_CACHE_DIR, exist_ok=True)
        for cpath in cpaths:
            tmp = cpath + ".tmp"
            _shutil.copy(neff_path, tmp)
            _os.replace(tmp, cpath)
    except Exception:
        pass
    return neff_path


_bass2jax.compile_bir_kernel = _cached_compile_bir_kernel
_bass_utils_mod.compile_bir_kernel = _cached_compile_bir_kernel
# ---------------------------------------------------------------------------

NCORES = 8
B, F, H, O, E, N_NODES = 2048, 512, 256, 1, 128, 4096
BL = B // NCORES          # samples per core = 256
NBT = BL // 128           # b-tiles per core = 2
EP = E // 2               # e-pairs = 64
KI = F // 128             # contraction chunks over input features = 4

F32 = mybir.dt.float32
BF16 = mybir.dt.bfloat16
MULT = mybir.AluOpType.mult
ADD = mybir.AluOpType.add
MAX = mybir.AluOpType.max
RELU = mybir.ActivationFunctionType.Relu
COPY = mybir.ActivationFunctionType.Copy

# tunables (validated against the deterministic cost-model timeline)
NWARM = 26                # warm-up matmuls (128-col) before real data lands
NFILL1 = 0                # filler matmuls between ep0 and ep1 (dl1 DMA wait)
PSUM_DIRECT_FROM = 59     # eps >= this skip ACT evac; DVE reads PSUM halves
DL_BUFS = 7               # dl tile pool depth

_CACHE = {}


def _build_nc():
    nc = bass.Bass()
    # pre-tiled operands: every DMA descriptor is a full >=512B SBUF row
    hd0_d = nc.dram_tensor("hd0", (128, KI * 128 + KI * H), BF16,
                           kind="ExternalInput")
    xtb_d = nc.dram_tensor("xtb", (128, KI * 128), BF16, kind="ExternalInput")
    dl_d = nc.dram_tensor("dl", (EP, F, 2 * H), BF16, kind="ExternalInput")
    r0_d = nc.dram_tensor("r0", (128, KI * H), BF16, kind="ExternalInput")
    cols_d = nc.dram_tensor("cols", (BL, E), F32, kind="ExternalInput")
    colse_d = nc.dram_tensor("colse", (E, BL), BF16, kind="ExternalInput")
    d1t_d = nc.dram_tensor("d1t", (E, H), BF16, kind="ExternalInput")
    r1t_d = nc.dram_tensor("r1t", (1, H), BF16, kind="ExternalInput")
    out_d = nc.dram_tensor("out", (BL, 1), F32, kind="ExternalOutput")

    with TileContext(nc) as tc:
        with (
            tc.tile_pool(name="const", bufs=1) as cpool,
            tc.tile_pool(name="acc", bufs=NBT) as apool,
            tc.tile_pool(name="dl", bufs=DL_BUFS) as dpool,
            tc.tile_pool(name="psum", bufs=3, space="PSUM") as ppool,
            tc.tile_pool(name="stage", bufs=8) as spool,
            tc.tile_pool(name="misc", bufs=4) as mpool,
        ):
            # --- resident tiles -------------------------------------------
            # head: [xta (k,b) | dl0 (k,n)] fused so one DMA carries the
            # first x half plus dl0-k01 (one HWDGE gen on the critical head)
            head = cpool.tile([128, KI * 128 + KI * 2 * H], BF16, tag="head")
            xt_sb = cpool.tile([128, KI * 128], BF16, tag="xt")
            r0_sb = cpool.tile([128, KI * H], BF16, tag="r0")
            cols_sb = cpool.tile([128, NBT * E], F32, tag="cols")
            colse_sb = cpool.tile([128, BL], BF16, tag="colse")
            d1t_sb = cpool.tile([128, H], BF16, tag="d1t")
            r1t_sb = cpool.tile([128, H], BF16, tag="r1t")

            dlts = [dpool.tile([128, KI * 2 * H], BF16, tag="dl",
                               name=f"dlt{ep}")
                    for ep in range(EP)]

            def dl_rhs(ep, lo, hi):
                if ep == 0:
                    off = KI * 128
                    return head[:, off + lo : off + hi]
                return dlts[ep][:, lo:hi]

            def dma_dl(ep, half=None):
                # half=None: whole e-pair tile; half=0/1: k01 / k23 halves
                if ep == 0:
                    # k23 half into the fused head tile (k01 rides with hd0)
                    assert half == 1
                    nc.sync.dma_start(
                        head[:, KI * 128 + KI * H :]
                        .rearrange("p (k n) -> p k n", k=KI // 2),
                        dl_d[0, 256:].rearrange("(k p) n -> p k n", p=128),
                    )
                    return
                if half is None:
                    nc.sync.dma_start(
                        dlts[ep][:].rearrange("p (k n) -> p k n", k=KI),
                        dl_d[ep].rearrange("(k p) n -> p k n", p=128),
                    )
                else:
                    sl = slice(half * 256, half * 256 + 256)
                    nc.sync.dma_start(
                        dlts[ep][:, half * KI * H * 2 // 2:
                                 half * KI * H * 2 // 2 + KI * H]
                        .rearrange("p (k n) -> p k n", k=KI // 2),
                        dl_d[ep, sl].rearrange("(k p) n -> p k n", p=128),
                    )

            # --- DMA issue order (critical-path tuned) --------------------
            nc.sync.dma_start(head[:, : KI * 128 + KI * H], hd0_d[:])
            dma_dl(0, 1)
            nc.sync.dma_start(xt_sb[:], xtb_d[:])
            for half in range(2):
                dma_dl(1, half)
            dma_dl(2)
            nc.sync.dma_start(r0_sb[:, : KI * H // 2], r0_d[:, : KI * H // 2])
            nc.sync.dma_start(r0_sb[:, KI * H // 2 :], r0_d[:, KI * H // 2 :])
            nc.sync.dma_start(
                cols_sb[:].rearrange("p (t n) -> p t n", t=NBT),
                cols_d[:].rearrange("(t p) n -> p t n", p=128),
            )
            for ep in (3, 4, 5):
                dma_dl(ep)
            nc.sync.dma_start(colse_sb[:], colse_d[:])
            nc.sync.dma_start(d1t_sb[:], d1t_d[:])
            nc.sync.dma_start(r1t_sb[:1, :], r1t_d[:])
            for ep in range(6, EP):
                dma_dl(ep)

            ones_sb = cpool.tile([128, 128], BF16, tag="ones")

            # PE warm-up: dummy matmuls on a zeroed scratch tile, issued with
            # no DMA dependency, keep the PE clock-ramp window open while the
            # first data transfers are in flight. The scratch PSUM is never
            # read.
            warm = cpool.tile([128, 128], BF16, tag="warm")
            nc.vector.memset(warm[:], 0.0)
            nc.gpsimd.memset(ones_sb[:1, :], 1.0)
            wps = ppool.tile([128, H], F32, tag="ps_l", name="wps", bufs=5)
            for w in range(NWARM):
                nc.tensor.matmul(
                    wps[:, :128], warm[:], warm[:], start=(w == 0),
                    stop=(w == NWARM - 1),
                )

            def x_lhsT(k, bt):
                # stationary operand: x^T chunk [128 (i), 128 (b)]
                base = head if bt == 0 else xt_sb
                return base[:, k * 128 : k * 128 + 128]

            accs = [None, None]
            wts = [None, None]
            res = mpool.tile([128, NBT], F32, tag="res")
            resr = mpool.tile([128, NBT], F32, tag="resr")

            def emit_root(bt):
                # acc[bt] = x @ root0, evacuated straight into acc
                ps = ppool.tile([128, H], F32, tag="ps_l",
                                name=f"ps_r{bt}", bufs=5)
                for k in range(KI):
                    nc.tensor.matmul(
                        ps[:],
                        x_lhsT(k, bt),
                        r0_sb[:, k * H : (k + 1) * H],
                        start=(k == 0),
                        stop=(k == KI - 1),
                    )
                acc = apool.tile([128, H], F32, tag="acc", name=f"acc{bt}")
                nc.scalar.activation(acc[:], ps[:], COPY)
                accs[bt] = acc

            def emit_wts(bt):
                # wt[b,h] = sum_e cols[b,e]*delta1[0,h,e] + root1[h]
                #         = colsE^T @ delta1[0]^T (K=E)  +  ones^T @ root1^T
                psw = ppool.tile([128, H], F32, tag="ps_l",
                                 name=f"ps_w{bt}", bufs=5)
                nc.tensor.matmul(
                    psw[:],
                    colse_sb[:, bt * 128 : (bt + 1) * 128],
                    d1t_sb[:],
                    start=True,
                    stop=False,
                )
                nc.tensor.matmul(
                    psw[:], ones_sb[:1, :], r1t_sb[:1, :], start=False, stop=True
                )
                wt = cpool.tile([128, H], F32, tag=f"wt{bt}")
                nc.scalar.activation(wt[:], psw[:], COPY)
                wts[bt] = wt

            # root / layer-2 weights: emitted here in program order (acc must
            # be initialized before the first STT) — the tile scheduler sinks
            # the matmuls to when r0/colse actually land
            emit_root(0)
            emit_root(1)
            emit_wts(0)
            emit_wts(1)

            # --- stage 1: stream delta, accumulate scaled matmuls ---------
            # PE: ps = x @ [W_{2ep} | W_{2ep+1}] ; ACT: evacuate PSUM->SBUF;
            # DVE: acc = stage_half * colsT[:, e] + acc. bt0 slots of the
            # last eps (and bt0's layer 2) run on the idle Pool engine so the
            # DVE is free for bt1's tail chain. For the last e-pair, bt1 uses
            # per-edge PSUM banks read directly by the DVE (no ACT hop).
            for ep in range(EP):
                last = ep == EP - 1
                direct = ep >= PSUM_DIRECT_FROM
                for bt in range(NBT):
                    if not direct:
                        ps = ppool.tile([128, 2 * H], F32, tag="ps")
                        for k in range(KI):
                            nc.tensor.matmul(
                                ps[:],
                                x_lhsT(k, bt),
                                dl_rhs(ep, k * 2 * H, (k + 1) * 2 * H),
                                start=(k == 0),
                                stop=(k == KI - 1),
                            )
                        stage = spool.tile([128, 2 * H], F32, tag="stage")
                        nc.scalar.activation(stage[:], ps[:], COPY)
                        halves = [stage[:, :H], stage[:, H:]]
                    else:
                        # per-edge PSUM banks; DVE reads PSUM directly and
                        # the first edge's STT starts 4 matmuls early
                        halves = []
                        for half in range(2):
                            psh = ppool.tile([128, H], F32, tag="ps_l",
                                             name=f"ps_l{bt}{half}", bufs=5)
                            for k in range(KI):
                                nc.tensor.matmul(
                                    psh[:],
                                    x_lhsT(k, bt),
                                    dl_rhs(ep, k * 2 * H + half * H,
                                           k * 2 * H + half * H + H),
                                    start=(k == 0),
                                    stop=(k == KI - 1),
                                )
                            halves.append(psh[:])
                    for half in range(2):
                        e = 2 * ep + half
                        col_ap = cols_sb[:, bt * E + e : bt * E + e + 1]
                        nc.vector.scalar_tensor_tensor(
                            out=accs[bt][:],
                            in0=halves[half],
                            scalar=col_ap,
                            in1=accs[bt][:],
                            op0=MULT,
                            op1=ADD,
                        )
                    if last:
                        # layer 2 for this bt immediately after its last STT
                        junk = mpool.tile([128, H], F32, tag="junk")
                        nc.vector.scalar_tensor_tensor(
                            out=junk[:],
                            in0=accs[bt][:],
                            scalar=0.0,
                            in1=wts[bt][:],
                            op0=MAX,
                            op1=MULT,
                            accum_out=res[:, bt : bt + 1],
                        )
                if ep == 0:
                    # filler keeps the PE ramp hot while dl1 is in flight
                    for _ in range(NFILL1):
                        nc.tensor.matmul(wps[:, :128], warm[:], warm[:],
                                         start=True, stop=True)

            # --- final relu + single combined output DMA ------------------
            # (A prepared SWDGE writeback + trigger would take the 625ns
            # HWDGE gen + 650ns DGE delay off this tail chain, but this
            # walrus build cannot codegen InstTriggerDma: "ISA wrong length".)
            nc.vector.tensor_scalar(resr[:], res[:], 0.0, None, MAX)
            nc.sync.dma_start(
                out_d[:].rearrange("(t p) o -> p (t o)", p=128), resr[:]
            )
    # Surgery on the prepared-writeback protocol:
    # 1. Strip the scheduler's data wait from the PREP: descriptor generation
    #    reads only addresses + the idx tile (Pool-local wait kept); data
    #    ordering vs the relu is enforced on the TRIGGER (step 3).
    # 2. The drain waits on the SWDGE proc sem (DMASW0), which in the
    #    prepared path is never ticked (the completion sem baked into the
    #    descriptors is out_dma) — rewrite those waits to out_dma.
    # 3. Make the TRIGGER wait for the relu via the relu's existing DVE
    #    engine-tick semaphore update (no extra sync commands needed).
    odma_num = out_dma_sem.num
    upd_val = None
    relu_tick = None
    _sem_cum = {}
    for f in nc.m.functions:
        for bb in f.blocks:
            for inst in bb.instructions:
                if inst.sync_info is None:
                    continue
                si = inst.sync_info
                if inst.name == prep_name:
                    si.on_wait = [w for w in si.on_wait
                                  if (w.ant_name or "").startswith("Pool")]
                    for u in si.on_update:
                        if u.id == odma_num:
                            upd_val = u.update_value
                elif inst.name == relu_name:
                    for u in si.on_update:
                        if (u.ant_name or "").startswith("DVE"):
                            cum = _sem_cum.get(u.id, 0) + (u.update_value or 1)
                            relu_tick = (u.id, u.ant_name, cum)
                for u in si.on_update:
                    _sem_cum[u.id] = _sem_cum.get(u.id, 0) + (u.update_value or 1)
    assert upd_val is not None, "out_dma update not found on prep"
    assert relu_tick is not None, "relu DVE tick update not found"
    # The trigger is a fixed-length InstISA: it cannot carry sync waits.
    # Move its waits (plus the new relu-tick wait) onto a NoOp inserted
    # right before it on the Pool engine (the wait-cap legalizer below
    # splits multi-wait NoOps as needed).
    for f in nc.m.functions:
        for bb in f.blocks:
            out = []
            for inst in bb.instructions:
                if inst.name == trig_name:
                    w = bass_rust.SyncWait(
                        sync_type="semaphore", id=relu_tick[0],
                        ant_name=relu_tick[1], wait_mode="sem-ge-imm",
                        wait_value=relu_tick[2],
                    )
                    nop = bass_rust.InstNoOp(
                        name=f"{inst.name}-tw", engine=inst.engine,
                        ins=[], outs=[],
                    )
                    nop.sync_info = bass_rust.SyncInfo(
                        on_wait=[w], on_update=[])
                    out.append(nop)
                out.append(inst)
            bb.instructions = out
    for f in nc.m.functions:
        for bb in f.blocks:
            for inst in bb.instructions:
                si = inst.sync_info
                if si is None:
                    continue
                waits = list(si.on_wait)
                changed = False
                for i, w in enumerate(waits):
                    if (w.ant_name or "").startswith("DMASW"):
                        waits[i] = bass_rust.SyncWait(
                            sync_type="semaphore", id=odma_num,
                            ant_name="out_dma", wait_mode="sem-ge-imm",
                            wait_value=upd_val,
                        )
                        changed = True
                if changed:
                    si.on_wait = waits
    _legalize_wait_counts(nc)
    return nc


def _prep_inputs(x, node_idx, path_mat, root0, root1, delta0, delta1):
    bf16 = ml_dtypes.bfloat16
    x = np.asarray(x, np.float32)
    path_mat = np.asarray(path_mat, np.float32)
    root0 = np.asarray(root0, np.float32)
    root1 = np.asarray(root1, np.float32)
    delta0 = np.asarray(delta0, np.float32)
    delta1 = np.asarray(delta1, np.float32)
    colsT = path_mat.T[np.asarray(node_idx, dtype=np.int64)].astype(np.float32)
    # delta0 (H, F, E) -> (E, F, H) -> pair e's along the free axis
    dt_ = np.ascontiguousarray(delta0.transpose(2, 1, 0))
    dl = np.ascontiguousarray(
        dt_.reshape(EP, 2, F, H).transpose(0, 2, 1, 3)
    ).reshape(EP, F, 2 * H)
    dl16 = dl.astype(bf16)
    xT16 = np.ascontiguousarray(x.T).astype(bf16)  # (F, B)
    # pre-tile r0 to [128 (i%128), (k, h)] so DMA rows are contiguous
    r0t = np.ascontiguousarray(
        root0.reshape(KI, 128, H).transpose(1, 0, 2)
    ).reshape(128, KI * H).astype(bf16)
    colsE16 = np.ascontiguousarray(colsT.T).astype(bf16)  # (E, B)
    d1t = np.ascontiguousarray(delta1[0].T).astype(bf16)  # (E, H)
    r1t = np.ascontiguousarray(root1.T).astype(bf16)  # (1, H)

    in_maps = []
    for c in range(NCORES):
        sl = slice(c * BL, (c + 1) * BL)
        xc = xT16[:, sl]  # (F, BL)
        # pre-tile x^T per bt-half to [128 (i%128), (k, b)]
        xparts = []
        for bt in range(NBT):
            xb = xc[:, bt * 128 : (bt + 1) * 128]  # (F, 128)
            xparts.append(
                np.ascontiguousarray(
                    xb.reshape(KI, 128, 128).transpose(1, 0, 2)
                ).reshape(128, KI * 128)
            )
        dl0k01 = np.ascontiguousarray(
            np.asarray(dl16[0][:256]).reshape(2, 128, 2 * H)
            .transpose(1, 0, 2)
        ).reshape(128, KI * H)
        hd0 = np.concatenate([np.asarray(xparts[0]), dl0k01], axis=1)
        in_maps.append(
            {
                "hd0": np.ascontiguousarray(hd0),
                "xtb": xparts[1],
                "dl": dl16,
                "r0": r0t,
                "cols": np.ascontiguousarray(colsT[sl]),
                "colse": np.ascontiguousarray(colsE16[:, sl]),
                "d1t": d1t,
                "r1t": r1t,
            }
        )
    return in_maps


def _run(inputs, trace=False, **kw):
    if "nc" not in _CACHE:
        _CACHE["nc"] = _build_nc()
    nc = _CACHE["nc"]
    in_maps = _prep_inputs(**inputs)
    res = run_bass_kernel_spmd(
        nc, in_maps, core_ids=list(range(NCORES)), trace=trace, **kw
    )
    out = np.concatenate([r["out"][:, 0] for r in res.results]).astype(np.float32)
    return out, res


def kernel(**inputs) -> np.ndarray:
    out, _ = _run(inputs)
    return out

